# revision 3
# baseline (speedup 1.0000x reference)
"""MAF (5-layer MADE + BatchNorm) forward pass on 8 Trainium2 NeuronCores.

Strategy (pure data parallel, feature-major):
  - Shard batch (16384) across 8 cores -> 2048 rows/core.
  - On-device layout is feature-major: features on SBUF partitions, batch on
    the free dim.  Weights are used as matmul lhsT in natural [in,out] layout
    (no activation transposes anywhere).
  - D=784 padded to 896 (7 tiles of 128).  Masks are pre-multiplied into the
    weights on the host; bias/BN vectors become per-partition operands.
  - logdet's sum over alpha features is folded into one extra output column
    of the (masked) alpha weight block: column 800 of the padded alpha block
    holds rowsums, so PSUM partition 32 of the last alpha tile carries
    sum_f alpha_raw[f, b] for free.
  - Matmuls run in float32r (TF32-like, full PE speed at free dim 512).
    Everything that feeds a matmul (x tiles, h1, h2, weights) is float32r;
    the fp32 accumulate happens in PSUM.
  - Weights are streamed per (layer, chunk-half) m-block-contiguously from
    DRAM; x stays resident in SBUF for the whole kernel.

Per core loop: for layer i (5) / chunk-half ch (2x1024 batch cols):
  part1: h1 = relu(W0m.T @ x + b0)       (8 m-tiles x 2 subs x 7 k-tiles)
  part2: h2 = relu(W1m.T @ h1 + b1)      (8 x 2 x 8)
  part3: per feature-tile f: alpha tile -> e = exp(-alpha-b2a), mu tile ->
         x = ((x - b2mu) - mu_raw) * e, then BN affine (layers 0-3) or
         store to z (layer 4).  logdet accumulated from the alpha-sum row.
"""

import sys

for _p in ('/opt/trn_rl_repo/concourse', '/opt/trn_rl_repo'):
    if _p not in sys.path:
        sys.path.insert(0, _p)

import numpy as np

import concourse.bass as bass  # noqa: F401  (import needed for side effects)
import concourse.mybir as mybir
import concourse.tile as tile
from concourse import bacc, bass_utils

# Problem constants (hardcoded per contract)
D = 784
Dp = 896          # padded feature dim, 7 tiles of 128
KD = 7            # x feature tiles
H = 1024
KH = 8            # h feature tiles
L = 5
B = 16384
NCORES = 8
BC = B // NCORES  # 2048 batch rows per core
CH = 2            # chunk halves per core (1024 batch cols each)
CHW = BC // CH    # 1024
SUB = 2           # 512-wide sub-chunks per chunk half
N = CHW // SUB    # 512 matmul free dim
EPS = 1e-5
SUMCOL = 800      # padded alpha-block column holding the rowsum (tile 6, part 32)

F32 = mybir.dt.float32
F32R = mybir.dt.float32r
MM_DT = F32   # matmul operand dtype: F32 (accurate) or F32R (fast, TF32-like)

_CACHE = {}


def _build_bass():
    """Build + compile the (input-agnostic) Bass program once."""
    if 'nc' in _CACHE:
        return _CACHE['nc']

    nc = bacc.Bacc("TRN2", target_bir_lowering=False, debug=False,
                   num_devices=NCORES)

    xT_d = nc.dram_tensor("xT", [128, KD, BC], MM_DT, kind="ExternalInput").ap()
    W0_d = nc.dram_tensor("W0L", [L, KH, 128, KD, 128], MM_DT, kind="ExternalInput").ap()
    W1_d = nc.dram_tensor("W1L", [L, KH, 128, KH, 128], MM_DT, kind="ExternalInput").ap()
    W2_d = nc.dram_tensor("W2L", [L, 2 * KD, 128, KH, 128], MM_DT, kind="ExternalInput").ap()
    bias_d = nc.dram_tensor("biases", [128, 206], F32, kind="ExternalInput").ap()

    zT_d = nc.dram_tensor("zT", [128, KD, BC], F32, kind="ExternalOutput").ap()
    ld_d = nc.dram_tensor("ld", [1, BC], F32, kind="ExternalOutput").ap()

    # bias column offsets within the consolidated [128, 206] tile
    OB0, OB1, OMU, OAL, OG, OBB = 0, 40, 80, 115, 150, 178

    AF = mybir.ActivationFunctionType
    ALU = mybir.AluOpType

    with tile.TileContext(nc) as tc:
        with tc.tile_pool(name="persist", bufs=1) as persist, \
             tc.tile_pool(name="w0p", bufs=3) as w0p, \
             tc.tile_pool(name="w1p", bufs=3) as w1p, \
             tc.tile_pool(name="w2p", bufs=3) as w2p, \
             tc.tile_pool(name="h1p", bufs=1) as h1p, \
             tc.tile_pool(name="h2p", bufs=1) as h2p, \
             tc.tile_pool(name="ep", bufs=3) as ep, \
             tc.tile_pool(name="tmp", bufs=4) as tmpp, \
             tc.tile_pool(name="zp", bufs=3) as zp, \
             tc.tile_pool(name="ps", bufs=7, space="PSUM") as psp:

            bias = persist.tile([128, 206], F32)
            nc.sync.dma_start(bias[:], bias_d)

            ld = persist.tile([1, BC], F32)
            nc.vector.memset(ld[:], 0.0)

            # resident x state: one tile per (feature-tile, chunk-half)
            xt = [[persist.tile([128, CHW], MM_DT, name=f"x_{k}_{c}")
                   for c in range(CH)] for k in range(KD)]
            for k in range(KD):
                for c in range(CH):
                    nc.sync.dma_start(xt[k][c][:], xT_d[:, k, c * CHW:(c + 1) * CHW])

            for i in range(L):
                for c in range(CH):
                    # ---- part 1: h1 = relu(W0m.T @ x + b0) ----
                    h1 = h1p.tile([128, KH, CHW], MM_DT, tag="h1")
                    for m in range(KH):
                        w = w0p.tile([128, KD, 128], MM_DT, tag="w0")
                        nc.sync.dma_start(w[:], W0_d[i, m])
                        for s in range(SUB):
                            ps = psp.tile([128, N], F32, tag="ps")
                            for k in range(KD):
                                nc.tensor.matmul(
                                    ps[:], w[:, k], xt[k][c][:, s * N:(s + 1) * N],
                                    start=(k == 0), stop=(k == KD - 1))
                            nc.scalar.activation(
                                h1[:, m, s * N:(s + 1) * N], ps[:], AF.Relu,
                                bias=bias[:, OB0 + i * 8 + m:OB0 + i * 8 + m + 1],
                                scale=1.0)

                    # ---- part 2: h2 = relu(W1m.T @ h1 + b1) ----
                    h2 = h2p.tile([128, KH, CHW], MM_DT, tag="h2")
                    for m in range(KH):
                        w = w1p.tile([128, KH, 128], MM_DT, tag="w1")
                        nc.sync.dma_start(w[:], W1_d[i, m])
                        for s in range(SUB):
                            ps = psp.tile([128, N], F32, tag="ps")
                            for k in range(KH):
                                nc.tensor.matmul(
                                    ps[:], w[:, k], h1[:, k, s * N:(s + 1) * N],
                                    start=(k == 0), stop=(k == KH - 1))
                            nc.scalar.activation(
                                h2[:, m, s * N:(s + 1) * N], ps[:], AF.Relu,
                                bias=bias[:, OB1 + i * 8 + m:OB1 + i * 8 + m + 1],
                                scale=1.0)

                    # ---- part 3: coupling update per feature tile ----
                    for f in range(KD):
                        wa = w2p.tile([128, KH, 128], MM_DT, tag="w2")
                        nc.sync.dma_start(wa[:], W2_d[i, KD + f])  # alpha block
                        wm = w2p.tile([128, KH, 128], MM_DT, tag="w2")
                        nc.sync.dma_start(wm[:], W2_d[i, f])       # mu block
                        e = ep.tile([128, CHW], F32, tag="e")
                        for s in range(SUB):
                            sl = slice(s * N, (s + 1) * N)
                            psa = psp.tile([128, N], F32, tag="ps")
                            for k in range(KH):
                                nc.tensor.matmul(
                                    psa[:], wa[:, k], h2[:, k, sl],
                                    start=(k == 0), stop=(k == KH - 1))
                            # e = exp(-(raw_alpha) - b2a)
                            nc.scalar.activation(
                                e[:, sl], psa[:], AF.Exp,
                                bias=bias[:, OAL + i * 7 + f:OAL + i * 7 + f + 1],
                                scale=-1.0)
                            if f == KD - 1:
                                # logdet: subtract the alpha-sum row (part 32)
                                col = c * CHW + s * N
                                nc.vector.tensor_tensor(
                                    ld[0:1, col:col + N], ld[0:1, col:col + N],
                                    psa[32:33, :], ALU.subtract)
                                # keep pad rows of x exactly zero: the sum row
                                # of e may be huge/inf; zero it before the mul
                                nc.vector.memset(e[32:33, sl], 0.0)

                            psm = psp.tile([128, N], F32, tag="ps")
                            for k in range(KH):
                                nc.tensor.matmul(
                                    psm[:], wm[:, k], h2[:, k, sl],
                                    start=(k == 0), stop=(k == KH - 1))
                            # t = x - b2mu  (per-partition scalar add of -b2mu)
                            t = tmpp.tile([128, N], F32, tag="t")
                            nc.vector.tensor_scalar_add(
                                t[:], xt[f][c][:, sl],
                                bias[:, OMU + i * 7 + f:OMU + i * 7 + f + 1])
                            # t = t - mu_raw
                            nc.vector.tensor_tensor(t[:], t[:], psm[:], ALU.subtract)
                            if i < L - 1:
                                # t = t * e ; x = t * g + b  (BN affine)
                                nc.vector.tensor_tensor(t[:], t[:], e[:, sl], ALU.mult)
                                nc.vector.tensor_scalar(
                                    xt[f][c][:, sl], t[:],
                                    bias[:, OG + i * 7 + f:OG + i * 7 + f + 1],
                                    bias[:, OBB + i * 7 + f:OBB + i * 7 + f + 1],
                                    ALU.mult, ALU.add)
                            else:
                                z = zp.tile([128, N], F32, tag="z")
                                nc.vector.tensor_tensor(z[:], t[:], e[:, sl], ALU.mult)
                                nc.sync.dma_start(
                                    zT_d[:, f, c * CHW + s * N:c * CHW + (s + 1) * N],
                                    z[:])

            nc.sync.dma_start(ld_d, ld[:])

    nc.compile()
    _CACHE['nc'] = nc
    return nc


def _prep_inputs(x, W0, b0, W1, b1, W2, b2, mask0, mask1, mask2,
                 gamma, beta, run_mean, run_var):
    """Host-side: mask weights, pad, rearrange to m-block layout, shard x."""
    f32 = np.float32
    x = np.asarray(x, f32)
    W0 = np.asarray(W0, f32) * np.asarray(mask0, f32)
    W1 = np.asarray(W1, f32) * np.asarray(mask1, f32)
    W2 = np.asarray(W2, f32) * np.asarray(mask2, f32)
    b0 = np.asarray(b0, f32)
    b1 = np.asarray(b1, f32)
    b2 = np.asarray(b2, f32)
    gamma = np.asarray(gamma, f32)
    beta = np.asarray(beta, f32)
    run_mean = np.asarray(run_mean, f32)
    run_var = np.asarray(run_var, f32)

    # --- weights: pad to [Dp, ...] and rearrange to [L, m, p, kt, mc] ---
    W0p = np.zeros((L, Dp, H), f32)
    W0p[:, :D, :] = W0
    W0L = np.ascontiguousarray(
        W0p.reshape(L, KD, 128, KH, 128).transpose(0, 3, 2, 1, 4))

    W1L = np.ascontiguousarray(
        W1.reshape(L, KH, 128, KH, 128).transpose(0, 3, 2, 1, 4))

    W2mu = np.zeros((L, H, Dp), f32)
    W2mu[:, :, :D] = W2[:, :, :D]
    W2a = np.zeros((L, H, Dp), f32)
    W2a[:, :, :D] = W2[:, :, D:]
    W2a[:, :, SUMCOL] = W2[:, :, D:].sum(axis=2)   # rowsum -> logdet
    W2big = np.concatenate([W2mu, W2a], axis=2)    # [L, H, 2*Dp]
    W2L = np.ascontiguousarray(
        W2big.reshape(L, KH, 128, 2 * KD, 128).transpose(0, 3, 2, 1, 4))

    # --- consolidated bias/BN tile [128, 206] ---
    biases = np.zeros((128, 206), f32)
    biases[:, 0:40] = b0.reshape(L, KH, 128).transpose(2, 0, 1).reshape(128, 40)
    biases[:, 40:80] = b1.reshape(L, KH, 128).transpose(2, 0, 1).reshape(128, 40)
    b2mu_p = np.zeros((L, Dp), f32)
    b2mu_p[:, :D] = b2[:, :D]
    b2a_p = np.zeros((L, Dp), f32)
    b2a_p[:, :D] = b2[:, D:]
    biases[:, 80:115] = (-b2mu_p).reshape(L, KD, 128).transpose(2, 0, 1).reshape(128, 35)
    biases[:, 115:150] = (-b2a_p).reshape(L, KD, 128).transpose(2, 0, 1).reshape(128, 35)
    std = np.sqrt(run_var + EPS)
    g_p = np.ones((L - 1, Dp), f32)
    g_p[:, :D] = gamma / std
    bb_p = np.zeros((L - 1, Dp), f32)
    bb_p[:, :D] = beta - run_mean * gamma / std
    biases[:, 150:178] = g_p.reshape(L - 1, KD, 128).transpose(2, 0, 1).reshape(128, 28)
    biases[:, 178:206] = bb_p.reshape(L - 1, KD, 128).transpose(2, 0, 1).reshape(128, 28)

    # --- x: transpose to feature-major, pad, shard ---
    xpad = np.zeros((Dp, B), f32)
    xpad[:D, :] = x.T
    xTt = xpad.reshape(KD, 128, B).transpose(1, 0, 2)  # [128, KD, B]
    in_maps = []
    for cix in range(NCORES):
        xs = np.ascontiguousarray(xTt[:, :, cix * BC:(cix + 1) * BC])
        in_maps.append({"xT": xs, "W0L": W0L, "W1L": W1L, "W2L": W2L,
                        "biases": biases})

    # host constant folded into logdet:
    #   + sum_i<4 sum_f log|gamma/std|  - sum_i sum_f b2_alpha
    c_host = (np.sum(np.log(np.abs(gamma / std)), dtype=np.float64)
              - np.sum(b2[:, D:], dtype=np.float64))
    return in_maps, np.float32(c_host)


def _run(inputs, trace=False):
    nc = _build_bass()
    in_maps, c_host = _prep_inputs(**inputs)
    res = bass_utils.run_bass_kernel_spmd(
        nc, in_maps, core_ids=list(range(NCORES)), trace=trace)

    z = np.empty((B, D), np.float32)
    logdet = np.empty((B,), np.float32)
    for cix, r in enumerate(res.results):
        zT = r["zT"]                                   # [128, KD, BC]
        zc = zT.transpose(1, 0, 2).reshape(Dp, BC)[:D]  # [D, BC]
        z[cix * BC:(cix + 1) * BC] = zc.T
        logdet[cix * BC:(cix + 1) * BC] = r["ld"][0] + c_host
    return z, logdet, res


def kernel(**inputs):
    z, logdet, _ = _run(inputs)
    return z, logdet
